# revision 8
# baseline (speedup 1.0000x reference)
"""AttnBlock (GroupNorm -> qkv 1x1 -> NxN spatial attention -> proj -> residual)
for Trainium2, SPMD over 8 NeuronCores.

Sharding: core = (batch b in 0..3, query-half qh in 0..1). Each core computes
K/V for its whole batch (replicated across the pair) and attention + proj for
its 2048 of the 4096 query positions. The query half is selected on the host
by rotating the spatial columns of x so the core's queries are always columns
0..2047 of its input -- one SPMD program serves all 8 cores (key order is
irrelevant to softmax-attention, and the host un-rotates nothing because only
the query columns are written back).

On-chip layout: channels on partitions ([c, N], 4 chunks of 128). Scores are
computed transposed (S^T[j, i] = sum_c K[c,j] Q[c,i]) so that the attention
weights come out in the [j, i] layout that the AV and proj matmuls consume as
lhsT/rhs directly -- no on-chip transposes anywhere. Softmax is computed
without max-subtraction (logits are +-1.5 for this problem's 0.02-scaled
weights), the denominator is accumulated on DVE and reduced across partitions
with a ones-matmul, and the 1/denom normalization is deferred past the proj
matmul (it commutes through the channel contraction).

Matmul operands are bf16 (1 PE cycle/row vs 4 for fp32); accumulation is fp32
in PSUM; the GroupNorm statistics and the final residual add stay fp32.
"""

import numpy as np

_B, _C, _HW = 4, 512, 64 * 64  # batch, channels, spatial N
_N = _HW                       # 4096
_NQ = _N // 2                  # queries per core
_G = 32                        # groupnorm groups
_EPS = 1e-6
_NCORES = 8
_CCH = _C // 128               # 4 channel chunks

_cached = None  # (nc,) built Bass program, reused across kernel() calls


def _legalize_single_wait(nc, mybir):
    """This container's walrus codegen accepts at most ONE sync-wait per
    instruction. Tile emits N-wait instructions; hoist the extras onto
    injected same-engine NOPs placed immediately before."""
    ctr = 0
    for f in nc.m.functions:
        for bb in f.blocks:
            out = []
            changed = False
            for inst in bb.instructions:
                si = inst.sync_info
                if si is not None and len(si.on_wait) > 1:
                    waits = list(si.on_wait)
                    for w in waits[:-1]:
                        ctr += 1
                        out.append(mybir.InstNoOp(
                            name=f"I-legalize-wait-{ctr}",
                            engine=inst.engine,
                            sync_info=mybir.SyncInfo(on_wait=[w], on_update=[]),
                        ))
                    inst.sync_info = mybir.SyncInfo(
                        on_wait=[waits[-1]], on_update=list(si.on_update))
                    changed = True
                out.append(inst)
            if changed:
                bb.instructions = out


def _build_program():
    import concourse.bass as bass
    import concourse.tile as tile
    import concourse.mybir as mybir

    f32 = mybir.dt.float32
    bf16 = mybir.dt.bfloat16
    AF = mybir.ActivationFunctionType
    OP = mybir.AluOpType

    nc = bass.Bass(name="attnblock")

    xb16 = nc.declare_dram_parameter("xb16", [_C, _N], bf16, isOutput=False)
    xqf = nc.declare_dram_parameter("xqf", [_C, _NQ], f32, isOutput=False)
    wqT = nc.declare_dram_parameter("wqT", [128, _CCH * _C], bf16, isOutput=False)
    wkT = nc.declare_dram_parameter("wkT", [128, _CCH * _C], bf16, isOutput=False)
    wvT = nc.declare_dram_parameter("wvT", [128, _CCH * _C], bf16, isOutput=False)
    wpT = nc.declare_dram_parameter("wpT", [128, _CCH * _C], bf16, isOutput=False)
    # all small [128, x] constants packed into one tensor:
    # [o128(1) | obc(128) | bq2(4) | bk2(4) | bpe2(4) | gnw2(4) | gnb2(4) | gmat(8)]
    consts = nc.declare_dram_parameter("consts", [128, 157], f32, isOutput=False)
    gexp = nc.declare_dram_parameter("gexp", [8, 128], f32, isOutput=False)
    out_d = nc.declare_dram_parameter("out", [_C, _NQ], f32, isOutput=True)

    scale = float(_C) ** -0.5

    with tile.TileContext(nc) as tc:
        with (
            tc.tile_pool(name="singles", bufs=1) as singles,
            tc.tile_pool(name="persist", bufs=1) as persist,
            tc.tile_pool(name="stats", bufs=2) as stats_p,
        ):
            # ---- constants / weights -------------------------------------
            sb_consts = singles.tile([128, 157], f32, tag="consts")
            nc.sync.dma_start(out=sb_consts, in_=consts[:, :])
            sb_o128 = sb_consts[:, 0:1]
            sb_obc = sb_consts[:, 1:129]
            sb_bq = sb_consts[:, 129:133]
            sb_bk = sb_consts[:, 133:137]
            sb_bpe = sb_consts[:, 137:141]
            sb_gnw = sb_consts[:, 141:145]
            sb_gnb = sb_consts[:, 145:149]
            sb_gmat = sb_consts[:, 149:157]
            sb_gexp = singles.tile([8, 128], f32, tag="gexp")
            nc.scalar.dma_start(out=sb_gexp, in_=gexp[:, :])
            sb_eps8 = singles.tile([8, 1], f32, tag="eps8")
            nc.vector.memset(sb_eps8, _EPS)
            sb_warm = singles.tile([128, 1], f32, tag="warm1")
            nc.vector.memset(sb_warm, 1.0)

            # mu' and rstd' per channel, per chunk: [128, chunk, {mu, rstd}]
            musig = singles.tile([128, _CCH, 2], f32, tag="musig")

            # hn (normalized x, bf16) [c, N] in 4 chunk tiles
            hn_t = [persist.tile([128, _N], bf16, tag=f"hn{ci}", name=f"hn{ci}")
                    for ci in range(_CCH)]

            # ---- phase 1: GroupNorm --------------------------------------
            with (
                tc.tile_pool(name="gn_stream", bufs=1) as gn_stream,
                tc.tile_pool(name="gn_small", bufs=2) as gn_small,
                tc.tile_pool(name="gn_psum", bufs=2, space="PSUM") as gn_psum,
                tc.tile_pool(name="warm_psum", bufs=1, space="PSUM") as warm_psum,
            ):
                # x chunks, halves spread across the three DMA-capable
                # engines' queues for parallel transfer
                dma_engs = [nc.sync, nc.scalar, nc.gpsimd]
                xts = []
                k = 0
                for ci in range(_CCH):
                    xt = gn_stream.tile([128, _N], bf16, tag=f"xt{ci}",
                                        name=f"xt{ci}")
                    for h in range(4):
                        eng = dma_engs[k % 3]
                        k += 1
                        sl = slice(h * (_N // 4), (h + 1) * (_N // 4))
                        eng.dma_start(out=xt[:, sl],
                                      in_=xb16[ci * 128:(ci + 1) * 128, sl])
                    xts.append(xt)

                # weights load after x (needed ~15us later)
                w_tiles = {}
                w_engines = {"wq": nc.scalar, "wk": nc.gpsimd,
                             "wv": nc.scalar, "wp": nc.gpsimd}
                for nm, src in (("wq", wqT), ("wk", wkT), ("wv", wvT),
                                ("wp", wpT)):
                    t = singles.tile([128, _CCH, _C], bf16, tag=f"w_{nm}",
                                     name=f"w_{nm}")
                    w_engines[nm].dma_start(
                        out=t, in_=src.rearrange("p (a f) -> p a f", a=_CCH))
                    w_tiles[nm] = t

                # PE warm-up: the GroupNorm head leaves the tensor engine
                # mostly idle, which wastes the HAM ramp and would make the
                # projection phase run at 1.2 GHz. Feed it throwaway matmuls,
                # interleaved with the GN chain in program order so the GN
                # group-statistics matmuls are not queued behind them.
                warm_ps = warm_psum.tile([128, 512], f32, tag="warm")

                def warm(n_small, n_big):
                    for _ in range(n_small):
                        nc.tensor.matmul(warm_ps[0:1, 0:1], lhsT=sb_warm,
                                         rhs=sb_warm, start=True, stop=True)
                    for _ in range(n_big):
                        nc.tensor.matmul(warm_ps, lhsT=xts[0][:, 0:128],
                                         rhs=xts[0][:, 0:512],
                                         start=True, stop=True)

                warm(40, 20)
                for ci in range(_CCH):
                    xt = xts[ci]
                    # per-partition sum (DVE) and sum of squares (ACT,
                    # accumulated along the free dim; the Square results are
                    # scratch, dumped into the hn tile which the normalize
                    # below overwrites anyway)
                    s2 = gn_small.tile([128, 2], f32, tag="s2")
                    if ci % 2 == 0:
                        nc.vector.reduce_sum(out=s2[:, 0:1], in_=xt,
                                             axis=mybir.AxisListType.XYZW)
                    else:
                        # offload the bulk of the sum to GpSimd (3 pairwise
                        # adds) so DVE only reduces a quarter-width strip
                        ga = gn_small.tile([128, 1024], f32, tag="ga")
                        gb = gn_small.tile([128, 1024], f32, tag="gb")
                        nc.gpsimd.tensor_add(ga, xt[:, 0:1024], xt[:, 1024:2048])
                        nc.gpsimd.tensor_add(gb, xt[:, 2048:3072], xt[:, 3072:4096])
                        nc.gpsimd.tensor_add(ga, ga, gb)
                        nc.vector.reduce_sum(out=s2[:, 0:1], in_=ga,
                                             axis=mybir.AxisListType.XYZW)
                    nc.scalar.activation(out=hn_t[ci], in_=xt, func=AF.Square,
                                         accum_out=s2[:, 1:2])
                    # per-group [8, 2] = gmat.T @ [sum_p, sumsq_p] / 65536
                    pg = gn_psum.tile([8, 2], f32, tag="pg")
                    nc.tensor.matmul(pg, lhsT=sb_gmat, rhs=s2, start=True,
                                     stop=True)
                    gs = gn_small.tile([8, 2], f32, tag="gs")
                    nc.scalar.copy(out=gs, in_=pg)
                    # var_g = m2 - mu^2 ; rstd_g = 1/sqrt(var+eps)
                    musq = gn_small.tile([8, 1], f32, tag="musq")
                    nc.vector.tensor_mul(musq, gs[:, 0:1], gs[:, 0:1])
                    nc.vector.tensor_tensor(
                        out=gs[:, 1:2], in0=gs[:, 1:2], in1=musq,
                        op=OP.subtract)
                    sq8 = gn_small.tile([8, 1], f32, tag="sq8")
                    nc.scalar.activation(
                        out=sq8, in_=gs[:, 1:2], func=AF.Sqrt, bias=sb_eps8)
                    nc.vector.reciprocal(out=gs[:, 1:2], in_=sq8)
                    # broadcast to channels: [128, 2] = gexp.T @ [mu_g, rstd_g]
                    pc = gn_psum.tile([128, 2], f32, tag="pc")
                    nc.tensor.matmul(pc, lhsT=sb_gexp, rhs=gs, start=True,
                                     stop=True)
                    pcs = gn_small.tile([128, 2], f32, tag="pcs")
                    nc.scalar.copy(out=pcs, in_=pc)
                    # fold gamma/beta: rstd' = rstd*gamma ; mu' = mu - beta/rstd'
                    nc.vector.tensor_mul(
                        musig[:, ci, 1:2], pcs[:, 1:2], sb_gnw[:, ci:ci + 1])
                    rec = gn_small.tile([128, 1], f32, tag="rec")
                    nc.vector.reciprocal(out=rec, in_=musig[:, ci, 1:2])
                    bs = gn_small.tile([128, 1], f32, tag="bs")
                    nc.vector.tensor_mul(bs, sb_gnb[:, ci:ci + 1], rec)
                    nc.vector.tensor_tensor(
                        out=musig[:, ci, 0:1], in0=pcs[:, 0:1], in1=bs,
                        op=OP.subtract)
                    # hn = (x - mu') * rstd'   (bf16 out)
                    nc.vector.tensor_scalar(
                        out=hn_t[ci], in0=xt,
                        scalar1=musig[:, ci, 0:1], scalar2=musig[:, ci, 1:2],
                        op0=OP.subtract, op1=OP.mult)
                    warm(0, 15)

            # ---- phase 2: Q, K, V^T projections --------------------------
            k_t = [persist.tile([128, _N], bf16, tag=f"K{o}", name=f"K{o}") for o in range(_CCH)]
            q_t = [persist.tile([128, _NQ], bf16, tag=f"Q{o}", name=f"Q{o}") for o in range(_CCH)]
            vt_t = persist.tile([128, 32, _C], bf16, tag="VT")

            with (
                tc.tile_pool(name="kq_psum", bufs=2, space="PSUM") as kq_psum,
                tc.tile_pool(name="vt_psum", bufs=2, space="PSUM") as vt_psum,
            ):
                for o in range(_CCH):
                    osl = slice(o * 128, (o + 1) * 128)
                    # K[o]: j over full N, in 1024-wide groups
                    for jg in range(_N // 1024):
                        ps = kq_psum.tile([128, 2, 512], f32, tag="kq")
                        for jj in range(2):
                            j0 = jg * 1024 + jj * 512
                            for ci in range(_CCH):
                                nc.tensor.matmul(
                                    ps[:, jj, :],
                                    lhsT=w_tiles["wk"][:, ci, osl],
                                    rhs=hn_t[ci][:, j0:j0 + 512],
                                    start=(ci == 0), stop=(ci == _CCH - 1))
                        nc.scalar.activation(
                            out=k_t[o][:, jg * 1024:(jg + 1) * 1024],
                            in_=ps.rearrange("p a b -> p (a b)"),
                            func=AF.Identity, bias=sb_bk[:, o:o + 1])
                    # Q[o]: j over first NQ columns (the rotated query half),
                    # attention scale and bias*scale folded in here
                    for jg in range(_NQ // 1024):
                        ps = kq_psum.tile([128, 2, 512], f32, tag="kq")
                        for jj in range(2):
                            j0 = jg * 1024 + jj * 512
                            for ci in range(_CCH):
                                nc.tensor.matmul(
                                    ps[:, jj, :],
                                    lhsT=w_tiles["wq"][:, ci, osl],
                                    rhs=hn_t[ci][:, j0:j0 + 512],
                                    start=(ci == 0), stop=(ci == _CCH - 1))
                        nc.scalar.activation(
                            out=q_t[o][:, jg * 1024:(jg + 1) * 1024],
                            in_=ps.rearrange("p a b -> p (a b)"),
                            func=AF.Identity, bias=sb_bq[:, o:o + 1],
                            scale=scale)
                # V^T[j, c]: stationary = hn column slices
                for jc in range(32):
                    ps2 = vt_psum.tile([128, 512], f32, tag="vt")
                    for ci in range(_CCH):
                        nc.tensor.matmul(
                            ps2,
                            lhsT=hn_t[ci][:, jc * 128:(jc + 1) * 128],
                            rhs=w_tiles["wv"][:, ci, :],
                            start=(ci == 0), stop=(ci == _CCH - 1))
                    nc.scalar.copy(out=vt_t[:, jc, :], in_=ps2)

            # ---- phase 3: attention + proj + residual, per 512-query block
            with (
                tc.tile_pool(name="attw", bufs=1) as attw,
                tc.tile_pool(name="resw", bufs=3) as resw,
                tc.tile_pool(name="s_psum", bufs=2, space="PSUM") as s_psum,
                tc.tile_pool(name="o_psum", bufs=2, space="PSUM") as o_psum,
                tc.tile_pool(name="pd_psum", bufs=2, space="PSUM") as pd_psum,
            ):
                for ib in range(_NQ // 512):
                    isl = slice(ib * 512, (ib + 1) * 512)
                    es = attw.tile([128, 32, 512], bf16, tag="ES")
                    denom = attw.tile([128, 512], f32, tag="denom")
                    # scores^T + exp, 2 j-chunks (1024 wide) at a time
                    for jg in range(16):
                        ps = s_psum.tile([128, 2, 512], f32, tag="s")
                        for jj in range(2):
                            jc = jg * 2 + jj
                            for ci in range(_CCH):
                                nc.tensor.matmul(
                                    ps[:, jj, :],
                                    lhsT=k_t[ci][:, jc * 128:(jc + 1) * 128],
                                    rhs=q_t[ci][:, isl],
                                    start=(ci == 0), stop=(ci == _CCH - 1))
                        nc.scalar.activation(
                            out=es[:, jg * 2:(jg + 1) * 2, :].rearrange(
                                "p a b -> p (a b)"),
                            in_=ps.rearrange("p a b -> p (a b)"),
                            func=AF.Exp)
                        # accumulate softmax denominator partials (over j)
                        for jj in range(2):
                            jc = jg * 2 + jj
                            if jc == 0:
                                nc.vector.tensor_copy(out=denom, in_=es[:, 0, :])
                            else:
                                nc.vector.tensor_add(denom, denom, es[:, jc, :])
                    # O'^T[c, i] = sum_j V^T[j,c] * expS^T[j,i]  (unnormalized)
                    ot = attw.tile([128, _CCH, 512], bf16, tag="OT")
                    rbc_sb = attw.tile([128, 512], f32, tag="rbc")
                    for cc in range(_CCH):
                        pso = o_psum.tile([128, 512], f32, tag="o")
                        for jc in range(32):
                            nc.tensor.matmul(
                                pso,
                                lhsT=vt_t[:, jc, cc * 128:(cc + 1) * 128],
                                rhs=es[:, jc, :],
                                start=(jc == 0), stop=(jc == 31))
                        nc.scalar.copy(out=ot[:, cc, :], in_=pso)
                        if cc == 0:
                            # denominator: one fp32 matmul against an all-ones
                            # [128,128] stationary both reduces over partitions
                            # and broadcasts the sums to every partition row.
                            rbc = pd_psum.tile([128, 512], f32, tag="pd")
                            nc.tensor.matmul(rbc, lhsT=sb_obc, rhs=denom,
                                             start=True, stop=True)
                        if cc == 1:
                            # reciprocal emitted one AV group later so it runs
                            # well before the proj matmuls need rbc_sb.
                            nc.vector.reciprocal(out=rbc_sb, in_=rbc)
                    # proj + rescale + bias + residual
                    for oc in range(_CCH):
                        psp = pd_psum.tile([128, 512], f32, tag="pd")
                        for cc in range(_CCH):
                            nc.tensor.matmul(
                                psp,
                                lhsT=w_tiles["wp"][:, cc, oc * 128:(oc + 1) * 128],
                                rhs=ot[:, cc, :],
                                start=(cc == 0), stop=(cc == _CCH - 1))
                        xres = resw.tile([128, 512], f32, tag="xres")
                        nc.sync.dma_start(
                            out=xres, in_=xqf[oc * 128:(oc + 1) * 128, isl])
                        t1 = resw.tile([128, 512], f32, tag="t1")
                        nc.vector.tensor_tensor(
                            out=t1, in0=psp, in1=rbc_sb, op=OP.mult)
                        outt = resw.tile([128, 512], f32, tag="outt")
                        nc.vector.scalar_tensor_tensor(
                            out=outt, in0=t1, scalar=sb_bpe[:, oc:oc + 1],
                            in1=xres, op0=OP.add, op1=OP.add)
                        nc.sync.dma_start(
                            out=out_d[oc * 128:(oc + 1) * 128, isl], in_=outt)

    _legalize_single_wait(nc, mybir)
    return nc


def kernel(**inputs):
    import ml_dtypes
    from concourse.bass_utils import run_bass_kernel_spmd

    global _cached
    if _cached is None:
        _cached = _build_program()
    nc = _cached

    x = np.asarray(inputs["x"], dtype=np.float32)
    gn_w = np.asarray(inputs["gn_w"], dtype=np.float32)
    gn_b = np.asarray(inputs["gn_b"], dtype=np.float32)
    wq = np.asarray(inputs["wq"], dtype=np.float32)
    bq = np.asarray(inputs["bq"], dtype=np.float32)
    wk = np.asarray(inputs["wk"], dtype=np.float32)
    bk = np.asarray(inputs["bk"], dtype=np.float32)
    wv = np.asarray(inputs["wv"], dtype=np.float32)
    bv = np.asarray(inputs["bv"], dtype=np.float32)
    wp = np.asarray(inputs["wp"], dtype=np.float32)
    bp = np.asarray(inputs["bp"], dtype=np.float32)

    bf = ml_dtypes.bfloat16
    scale = float(_C) ** -0.5

    def cols(v):  # [512] -> [128, 4] chunk columns
        return np.ascontiguousarray(v.reshape(_CCH, 128).T)

    def wlay(w):  # [cout, cin] -> wT chunked as [128, cch*cout] contiguous
        return np.ascontiguousarray(
            w.T.reshape(_CCH, 128, _C).transpose(1, 0, 2).reshape(128, _CCH * _C)
        ).astype(bf)

    consts = np.concatenate([
        np.ones((128, 1), np.float32),                              # o128
        np.ones((128, 128), np.float32),                            # obc
        cols(bq * scale),                                           # bq2
        cols(bk),                                                   # bk2
        cols(wp @ bv + bp),                                         # bpe2
        cols(gn_w),                                                 # gnw2
        cols(gn_b),                                                 # gnb2
        np.repeat(np.eye(8, dtype=np.float32), 16, axis=0) / 65536.0,  # gmat
    ], axis=1)
    shared = {
        "wqT": wlay(wq),
        "wkT": wlay(wk),
        "wvT": wlay(wv),
        "wpT": wlay(wp),
        "consts": consts,
        "gexp": np.repeat(np.eye(8, dtype=np.float32), 16, axis=1),
    }

    xf = x.reshape(_B, _C, _N)
    in_maps = []
    for core in range(_NCORES):
        bi, qh = core // 2, core % 2
        xbc = xf[bi]
        if qh == 1:  # rotate so this core's queries are columns 0..NQ-1
            xbc = np.concatenate([xbc[:, _NQ:], xbc[:, :_NQ]], axis=1)
        in_maps.append({
            "xb16": np.ascontiguousarray(xbc).astype(bf),
            "xqf": np.ascontiguousarray(xbc[:, :_NQ], dtype=np.float32),
            **shared,
        })

    res = run_bass_kernel_spmd(nc, in_maps, core_ids=list(range(_NCORES)))

    out = np.empty((_B, _C, _N), np.float32)
    for core in range(_NCORES):
        bi, qh = core // 2, core % 2
        out[bi][:, qh * _NQ:(qh + 1) * _NQ] = res.results[core]["out"]
    return out.reshape(_B, _C, 64, 64)


# revision 9
# speedup vs baseline: 1.0046x; 1.0046x over previous
"""AttnBlock (GroupNorm -> qkv 1x1 -> NxN spatial attention -> proj -> residual)
for Trainium2, SPMD over 8 NeuronCores.

Sharding: core = (batch b in 0..3, query-half qh in 0..1). Each core computes
K/V for its whole batch (replicated across the pair) and attention + proj for
its 2048 of the 4096 query positions. The query half is selected on the host
by rotating the spatial columns of x so the core's queries are always columns
0..2047 of its input -- one SPMD program serves all 8 cores (key order is
irrelevant to softmax-attention, and the host un-rotates nothing because only
the query columns are written back).

On-chip layout: channels on partitions ([c, N], 4 chunks of 128). Scores are
computed transposed (S^T[j, i] = sum_c K[c,j] Q[c,i]) so that the attention
weights come out in the [j, i] layout that the AV and proj matmuls consume as
lhsT/rhs directly -- no on-chip transposes anywhere. Softmax is computed
without max-subtraction (logits are +-1.5 for this problem's 0.02-scaled
weights), the denominator is accumulated on DVE and reduced across partitions
with a ones-matmul, and the 1/denom normalization is deferred past the proj
matmul (it commutes through the channel contraction).

Matmul operands are bf16 (1 PE cycle/row vs 4 for fp32); accumulation is fp32
in PSUM; the GroupNorm statistics and the final residual add stay fp32.
"""

import numpy as np

_B, _C, _HW = 4, 512, 64 * 64  # batch, channels, spatial N
_N = _HW                       # 4096
_NQ = _N // 2                  # queries per core
_G = 32                        # groupnorm groups
_EPS = 1e-6
_NCORES = 8
_CCH = _C // 128               # 4 channel chunks

_cached = None  # (nc,) built Bass program, reused across kernel() calls


def _legalize_single_wait(nc, mybir):
    """This container's walrus codegen accepts at most ONE sync-wait per
    instruction. Tile emits N-wait instructions; hoist the extras onto
    injected same-engine NOPs placed immediately before."""
    ctr = 0
    for f in nc.m.functions:
        for bb in f.blocks:
            out = []
            changed = False
            for inst in bb.instructions:
                si = inst.sync_info
                if si is not None and len(si.on_wait) > 1:
                    waits = list(si.on_wait)
                    for w in waits[:-1]:
                        ctr += 1
                        out.append(mybir.InstNoOp(
                            name=f"I-legalize-wait-{ctr}",
                            engine=inst.engine,
                            sync_info=mybir.SyncInfo(on_wait=[w], on_update=[]),
                        ))
                    inst.sync_info = mybir.SyncInfo(
                        on_wait=[waits[-1]], on_update=list(si.on_update))
                    changed = True
                out.append(inst)
            if changed:
                bb.instructions = out


def _build_program():
    import concourse.bass as bass
    import concourse.tile as tile
    import concourse.mybir as mybir

    f32 = mybir.dt.float32
    bf16 = mybir.dt.bfloat16
    AF = mybir.ActivationFunctionType
    OP = mybir.AluOpType

    nc = bass.Bass(name="attnblock")

    xb16 = nc.declare_dram_parameter("xb16", [_C, _N], bf16, isOutput=False)
    xqf = nc.declare_dram_parameter("xqf", [_C, _NQ], f32, isOutput=False)
    wqT = nc.declare_dram_parameter("wqT", [128, _CCH * _C], bf16, isOutput=False)
    wkT = nc.declare_dram_parameter("wkT", [128, _CCH * _C], bf16, isOutput=False)
    wvT = nc.declare_dram_parameter("wvT", [128, _CCH * _C], bf16, isOutput=False)
    wpT = nc.declare_dram_parameter("wpT", [128, _CCH * _C], bf16, isOutput=False)
    # all small [128, x] constants packed into one tensor:
    # [o128(1) | obc(128) | bq2(4) | bk2(4) | bpe2(4) | gnw2(4) | gnb2(4) | gmat(8)]
    consts = nc.declare_dram_parameter("consts", [128, 157], f32, isOutput=False)
    gexp = nc.declare_dram_parameter("gexp", [8, 128], f32, isOutput=False)
    out_d = nc.declare_dram_parameter("out", [_C, _NQ], f32, isOutput=True)

    scale = float(_C) ** -0.5

    with tile.TileContext(nc) as tc:
        with (
            tc.tile_pool(name="singles", bufs=1) as singles,
            tc.tile_pool(name="persist", bufs=1) as persist,
            tc.tile_pool(name="stats", bufs=2) as stats_p,
        ):
            # ---- constants / weights -------------------------------------
            sb_consts = singles.tile([128, 157], f32, tag="consts")
            nc.sync.dma_start(out=sb_consts, in_=consts[:, :])
            sb_o128 = sb_consts[:, 0:1]
            sb_obc = sb_consts[:, 1:129]
            sb_bq = sb_consts[:, 129:133]
            sb_bk = sb_consts[:, 133:137]
            sb_bpe = sb_consts[:, 137:141]
            sb_gnw = sb_consts[:, 141:145]
            sb_gnb = sb_consts[:, 145:149]
            sb_gmat = sb_consts[:, 149:157]
            sb_gexp = singles.tile([8, 128], f32, tag="gexp")
            nc.sync.dma_start(out=sb_gexp, in_=gexp[:, :])
            sb_eps8 = singles.tile([8, 1], f32, tag="eps8")
            nc.vector.memset(sb_eps8, _EPS)
            sb_warm = singles.tile([128, 1], f32, tag="warm1")
            nc.vector.memset(sb_warm, 1.0)
            # touch Square and Exp so ACT_TABLE_LOAD happens during the DMA
            # head instead of on the GroupNorm critical path
            sb_actw = singles.tile([8, 2], f32, tag="actw")
            nc.scalar.activation(out=sb_actw[:, 0:1], in_=sb_eps8, func=AF.Square)
            nc.scalar.activation(out=sb_actw[:, 1:2], in_=sb_eps8, func=AF.Exp)

            # mu' and rstd' per channel, per chunk: [128, chunk, {mu, rstd}]
            musig = singles.tile([128, _CCH, 2], f32, tag="musig")

            # hn (normalized x, bf16) [c, N] in 4 chunk tiles
            hn_t = [persist.tile([128, _N], bf16, tag=f"hn{ci}", name=f"hn{ci}")
                    for ci in range(_CCH)]

            # ---- phase 1: GroupNorm --------------------------------------
            with (
                tc.tile_pool(name="gn_stream", bufs=1) as gn_stream,
                tc.tile_pool(name="gn_small", bufs=2) as gn_small,
                tc.tile_pool(name="gn_psum", bufs=2, space="PSUM") as gn_psum,
                tc.tile_pool(name="warm_psum", bufs=1, space="PSUM") as warm_psum,
            ):
                # x chunks, halves spread across the three DMA-capable
                # engines' queues for parallel transfer
                dma_engs = [nc.sync, nc.gpsimd]
                xts = []
                k = 0
                for ci in range(_CCH):
                    xt = gn_stream.tile([128, _N], bf16, tag=f"xt{ci}",
                                        name=f"xt{ci}")
                    for h in range(4):
                        eng = dma_engs[k % 2]
                        k += 1
                        sl = slice(h * (_N // 4), (h + 1) * (_N // 4))
                        eng.dma_start(out=xt[:, sl],
                                      in_=xb16[ci * 128:(ci + 1) * 128, sl])
                    xts.append(xt)

                # weights load after x (needed ~15us later); keep the scalar
                # engine free of DMA-trigger duty: it runs the Squares
                w_tiles = {}
                w_engines = {"wq": nc.sync, "wk": nc.gpsimd,
                             "wv": nc.sync, "wp": nc.gpsimd}
                for nm, src in (("wq", wqT), ("wk", wkT), ("wv", wvT),
                                ("wp", wpT)):
                    t = singles.tile([128, _CCH, _C], bf16, tag=f"w_{nm}",
                                     name=f"w_{nm}")
                    w_engines[nm].dma_start(
                        out=t, in_=src.rearrange("p (a f) -> p a f", a=_CCH))
                    w_tiles[nm] = t

                # PE warm-up: the GroupNorm head leaves the tensor engine
                # mostly idle, which wastes the HAM ramp and would make the
                # projection phase run at 1.2 GHz. Feed it throwaway matmuls,
                # interleaved with the GN chain in program order so the GN
                # group-statistics matmuls are not queued behind them.
                warm_ps = warm_psum.tile([128, 512], f32, tag="warm")

                def warm(n_small, n_big):
                    for _ in range(n_small):
                        nc.tensor.matmul(warm_ps[0:1, 0:1], lhsT=sb_warm,
                                         rhs=sb_warm, start=True, stop=True)
                    for _ in range(n_big):
                        nc.tensor.matmul(warm_ps, lhsT=xts[0][:, 0:128],
                                         rhs=xts[0][:, 0:512],
                                         start=True, stop=True)

                warm(40, 25)
                for ci in range(_CCH):
                    xt = xts[ci]
                    # per-partition sum (DVE) and sum of squares (ACT,
                    # accumulated along the free dim; the Square results are
                    # scratch, dumped into the hn tile which the normalize
                    # below overwrites anyway)
                    s2 = gn_small.tile([128, 2], f32, tag="s2")
                    nc.vector.reduce_sum(out=s2[:, 0:1], in_=xt,
                                         axis=mybir.AxisListType.XYZW)
                    nc.scalar.activation(out=hn_t[ci], in_=xt, func=AF.Square,
                                         accum_out=s2[:, 1:2])
                    # per-group [8, 2] = gmat.T @ [sum_p, sumsq_p] / 65536
                    pg = gn_psum.tile([8, 2], f32, tag="pg")
                    nc.tensor.matmul(pg, lhsT=sb_gmat, rhs=s2, start=True,
                                     stop=True)
                    gs = gn_small.tile([8, 2], f32, tag="gs")
                    nc.scalar.copy(out=gs, in_=pg)
                    # var_g = m2 - mu^2 ; rstd_g = 1/sqrt(var+eps)
                    musq = gn_small.tile([8, 1], f32, tag="musq")
                    nc.vector.tensor_mul(musq, gs[:, 0:1], gs[:, 0:1])
                    nc.vector.tensor_tensor(
                        out=gs[:, 1:2], in0=gs[:, 1:2], in1=musq,
                        op=OP.subtract)
                    sq8 = gn_small.tile([8, 1], f32, tag="sq8")
                    nc.scalar.activation(
                        out=sq8, in_=gs[:, 1:2], func=AF.Sqrt, bias=sb_eps8)
                    nc.vector.reciprocal(out=gs[:, 1:2], in_=sq8)
                    # broadcast to channels: [128, 2] = gexp.T @ [mu_g, rstd_g]
                    pc = gn_psum.tile([128, 2], f32, tag="pc")
                    nc.tensor.matmul(pc, lhsT=sb_gexp, rhs=gs, start=True,
                                     stop=True)
                    pcs = gn_small.tile([128, 2], f32, tag="pcs")
                    nc.scalar.copy(out=pcs, in_=pc)
                    # fold gamma/beta: rstd' = rstd*gamma ; mu' = mu - beta/rstd'
                    nc.vector.tensor_mul(
                        musig[:, ci, 1:2], pcs[:, 1:2], sb_gnw[:, ci:ci + 1])
                    rec = gn_small.tile([128, 1], f32, tag="rec")
                    nc.vector.reciprocal(out=rec, in_=musig[:, ci, 1:2])
                    bs = gn_small.tile([128, 1], f32, tag="bs")
                    nc.vector.tensor_mul(bs, sb_gnb[:, ci:ci + 1], rec)
                    nc.vector.tensor_tensor(
                        out=musig[:, ci, 0:1], in0=pcs[:, 0:1], in1=bs,
                        op=OP.subtract)
                    # hn = (x - mu') * rstd'   (bf16 out)
                    nc.vector.tensor_scalar(
                        out=hn_t[ci], in0=xt,
                        scalar1=musig[:, ci, 0:1], scalar2=musig[:, ci, 1:2],
                        op0=OP.subtract, op1=OP.mult)
                    warm(0, 25)

            # ---- phase 2: Q, K, V^T projections --------------------------
            k_t = [persist.tile([128, _N], bf16, tag=f"K{o}", name=f"K{o}") for o in range(_CCH)]
            q_t = [persist.tile([128, _NQ], bf16, tag=f"Q{o}", name=f"Q{o}") for o in range(_CCH)]
            vt_t = persist.tile([128, 32, _C], bf16, tag="VT")

            with (
                tc.tile_pool(name="kq_psum", bufs=2, space="PSUM") as kq_psum,
                tc.tile_pool(name="vt_psum", bufs=2, space="PSUM") as vt_psum,
            ):
                for o in range(_CCH):
                    osl = slice(o * 128, (o + 1) * 128)
                    # K[o]: j over full N, in 1024-wide groups
                    for jg in range(_N // 1024):
                        ps = kq_psum.tile([128, 2, 512], f32, tag="kq")
                        for jj in range(2):
                            j0 = jg * 1024 + jj * 512
                            for ci in range(_CCH):
                                nc.tensor.matmul(
                                    ps[:, jj, :],
                                    lhsT=w_tiles["wk"][:, ci, osl],
                                    rhs=hn_t[ci][:, j0:j0 + 512],
                                    start=(ci == 0), stop=(ci == _CCH - 1))
                        nc.scalar.activation(
                            out=k_t[o][:, jg * 1024:(jg + 1) * 1024],
                            in_=ps.rearrange("p a b -> p (a b)"),
                            func=AF.Identity, bias=sb_bk[:, o:o + 1])
                    # Q[o]: j over first NQ columns (the rotated query half),
                    # attention scale and bias*scale folded in here
                    for jg in range(_NQ // 1024):
                        ps = kq_psum.tile([128, 2, 512], f32, tag="kq")
                        for jj in range(2):
                            j0 = jg * 1024 + jj * 512
                            for ci in range(_CCH):
                                nc.tensor.matmul(
                                    ps[:, jj, :],
                                    lhsT=w_tiles["wq"][:, ci, osl],
                                    rhs=hn_t[ci][:, j0:j0 + 512],
                                    start=(ci == 0), stop=(ci == _CCH - 1))
                        nc.scalar.activation(
                            out=q_t[o][:, jg * 1024:(jg + 1) * 1024],
                            in_=ps.rearrange("p a b -> p (a b)"),
                            func=AF.Identity, bias=sb_bq[:, o:o + 1],
                            scale=scale)
                # V^T[j, c]: stationary = hn column slices
                for jc in range(32):
                    ps2 = vt_psum.tile([128, 512], f32, tag="vt")
                    for ci in range(_CCH):
                        nc.tensor.matmul(
                            ps2,
                            lhsT=hn_t[ci][:, jc * 128:(jc + 1) * 128],
                            rhs=w_tiles["wv"][:, ci, :],
                            start=(ci == 0), stop=(ci == _CCH - 1))
                    nc.scalar.copy(out=vt_t[:, jc, :], in_=ps2)

            # ---- phase 3: attention + proj + residual, per 512-query block
            with (
                tc.tile_pool(name="attw", bufs=1) as attw,
                tc.tile_pool(name="resw", bufs=3) as resw,
                tc.tile_pool(name="s_psum", bufs=2, space="PSUM") as s_psum,
                tc.tile_pool(name="o_psum", bufs=2, space="PSUM") as o_psum,
                tc.tile_pool(name="pd_psum", bufs=2, space="PSUM") as pd_psum,
            ):
                for ib in range(_NQ // 512):
                    isl = slice(ib * 512, (ib + 1) * 512)
                    es = attw.tile([128, 32, 512], bf16, tag="ES")
                    denom = attw.tile([128, 512], f32, tag="denom")
                    # scores^T + exp, 2 j-chunks (1024 wide) at a time
                    for jg in range(16):
                        ps = s_psum.tile([128, 2, 512], f32, tag="s")
                        for jj in range(2):
                            jc = jg * 2 + jj
                            for ci in range(_CCH):
                                nc.tensor.matmul(
                                    ps[:, jj, :],
                                    lhsT=k_t[ci][:, jc * 128:(jc + 1) * 128],
                                    rhs=q_t[ci][:, isl],
                                    start=(ci == 0), stop=(ci == _CCH - 1))
                        nc.scalar.activation(
                            out=es[:, jg * 2:(jg + 1) * 2, :].rearrange(
                                "p a b -> p (a b)"),
                            in_=ps.rearrange("p a b -> p (a b)"),
                            func=AF.Exp)
                        # accumulate softmax denominator partials (over j)
                        for jj in range(2):
                            jc = jg * 2 + jj
                            if jc == 0:
                                nc.vector.tensor_copy(out=denom, in_=es[:, 0, :])
                            else:
                                nc.vector.tensor_add(denom, denom, es[:, jc, :])
                    # O'^T[c, i] = sum_j V^T[j,c] * expS^T[j,i]  (unnormalized)
                    ot = attw.tile([128, _CCH, 512], bf16, tag="OT")
                    rbc_sb = attw.tile([128, 512], f32, tag="rbc")
                    for cc in range(_CCH):
                        pso = o_psum.tile([128, 512], f32, tag="o")
                        for jc in range(32):
                            nc.tensor.matmul(
                                pso,
                                lhsT=vt_t[:, jc, cc * 128:(cc + 1) * 128],
                                rhs=es[:, jc, :],
                                start=(jc == 0), stop=(jc == 31))
                        nc.scalar.copy(out=ot[:, cc, :], in_=pso)
                        if cc == 0:
                            # denominator: one fp32 matmul against an all-ones
                            # [128,128] stationary both reduces over partitions
                            # and broadcasts the sums to every partition row.
                            rbc = pd_psum.tile([128, 512], f32, tag="pd")
                            nc.tensor.matmul(rbc, lhsT=sb_obc, rhs=denom,
                                             start=True, stop=True)
                        if cc == 1:
                            # reciprocal emitted one AV group later so it runs
                            # well before the proj matmuls need rbc_sb.
                            nc.vector.reciprocal(out=rbc_sb, in_=rbc)
                    # proj + rescale + bias + residual
                    for oc in range(_CCH):
                        psp = pd_psum.tile([128, 512], f32, tag="pd")
                        for cc in range(_CCH):
                            nc.tensor.matmul(
                                psp,
                                lhsT=w_tiles["wp"][:, cc, oc * 128:(oc + 1) * 128],
                                rhs=ot[:, cc, :],
                                start=(cc == 0), stop=(cc == _CCH - 1))
                        xres = resw.tile([128, 512], f32, tag="xres")
                        nc.sync.dma_start(
                            out=xres, in_=xqf[oc * 128:(oc + 1) * 128, isl])
                        t1 = resw.tile([128, 512], f32, tag="t1")
                        nc.vector.tensor_tensor(
                            out=t1, in0=psp, in1=rbc_sb, op=OP.mult)
                        outt = resw.tile([128, 512], f32, tag="outt")
                        nc.vector.scalar_tensor_tensor(
                            out=outt, in0=t1, scalar=sb_bpe[:, oc:oc + 1],
                            in1=xres, op0=OP.add, op1=OP.add)
                        nc.sync.dma_start(
                            out=out_d[oc * 128:(oc + 1) * 128, isl], in_=outt)

    _legalize_single_wait(nc, mybir)
    return nc


def kernel(**inputs):
    import ml_dtypes
    from concourse.bass_utils import run_bass_kernel_spmd

    global _cached
    if _cached is None:
        _cached = _build_program()
    nc = _cached

    x = np.asarray(inputs["x"], dtype=np.float32)
    gn_w = np.asarray(inputs["gn_w"], dtype=np.float32)
    gn_b = np.asarray(inputs["gn_b"], dtype=np.float32)
    wq = np.asarray(inputs["wq"], dtype=np.float32)
    bq = np.asarray(inputs["bq"], dtype=np.float32)
    wk = np.asarray(inputs["wk"], dtype=np.float32)
    bk = np.asarray(inputs["bk"], dtype=np.float32)
    wv = np.asarray(inputs["wv"], dtype=np.float32)
    bv = np.asarray(inputs["bv"], dtype=np.float32)
    wp = np.asarray(inputs["wp"], dtype=np.float32)
    bp = np.asarray(inputs["bp"], dtype=np.float32)

    bf = ml_dtypes.bfloat16
    scale = float(_C) ** -0.5

    def cols(v):  # [512] -> [128, 4] chunk columns
        return np.ascontiguousarray(v.reshape(_CCH, 128).T)

    def wlay(w):  # [cout, cin] -> wT chunked as [128, cch*cout] contiguous
        return np.ascontiguousarray(
            w.T.reshape(_CCH, 128, _C).transpose(1, 0, 2).reshape(128, _CCH * _C)
        ).astype(bf)

    consts = np.concatenate([
        np.ones((128, 1), np.float32),                              # o128
        np.ones((128, 128), np.float32),                            # obc
        cols(bq * scale),                                           # bq2
        cols(bk),                                                   # bk2
        cols(wp @ bv + bp),                                         # bpe2
        cols(gn_w),                                                 # gnw2
        cols(gn_b),                                                 # gnb2
        np.repeat(np.eye(8, dtype=np.float32), 16, axis=0) / 65536.0,  # gmat
    ], axis=1)
    shared = {
        "wqT": wlay(wq),
        "wkT": wlay(wk),
        "wvT": wlay(wv),
        "wpT": wlay(wp),
        "consts": consts,
        "gexp": np.repeat(np.eye(8, dtype=np.float32), 16, axis=1),
    }

    xf = x.reshape(_B, _C, _N)
    in_maps = []
    for core in range(_NCORES):
        bi, qh = core // 2, core % 2
        xbc = xf[bi]
        if qh == 1:  # rotate so this core's queries are columns 0..NQ-1
            xbc = np.concatenate([xbc[:, _NQ:], xbc[:, :_NQ]], axis=1)
        in_maps.append({
            "xb16": np.ascontiguousarray(xbc).astype(bf),
            "xqf": np.ascontiguousarray(xbc[:, :_NQ], dtype=np.float32),
            **shared,
        })

    res = run_bass_kernel_spmd(nc, in_maps, core_ids=list(range(_NCORES)))

    out = np.empty((_B, _C, _N), np.float32)
    for core in range(_NCORES):
        bi, qh = core // 2, core % 2
        out[bi][:, qh * _NQ:(qh + 1) * _NQ] = res.results[core]["out"]
    return out.reshape(_B, _C, 64, 64)
